# revision 67
# baseline (speedup 1.0000x reference)
"""Trainium2 Bass kernel for the YOLO/FCOS-layer loss (nn_FCOSLayer_22840636080477).

Sharding: data-parallel over batch, 2 images per NeuronCore x 8 cores, one
SPMD program. Host does label-side preprocessing (anchor matching, scatter
dedup, row-band gt->partition scheduling, constant packing); device does
everything that touches `raw`:

  loss = sum_cells softplus(conf) * (conf_mask & ~gt)          (dense)
       + sum_gtcells [ softplus(conf)-conf                      (sparse, gather)
                      + sum_c (softplus(cls_c) - onehot_c*cls_c)
                      + sum_4 (ltrb_raw - tgt)^2 ]

The ignore mask needs a max-IoU scan of 12288 pred boxes against each
image's gt boxes.  Three structural tricks make it cheap:

1. bf16 + packed last-axis APs -> DVE 2x_1p mode (0.52 ns/elem).
2. Row banding: IoU > 0.6 forces the pred's cell center inside the gt
   box dilated by (2/3)*(wg,hg) (pred-independent bound), so each gt
   only concerns a contiguous band of partitions (partition p holds
   row p//2).  Interval-coloring packs gts into "rounds"; each round
   instruction gives every partition its own gt via per-partition
   consts.  Rounds ~ max-load instead of K.
3. Margin m = min(iw*ih - athg, iw) has the exact sign of the ignore
   condition (valid when gt heights <= 1), killing both clamps; the
   per-pred threshold athp = cthre*areap is compared once at the end:
   ignore <=> max_g m_g > athp.
"""
import sys
import math
import numpy as np

sys.path.insert(0, "/opt/trn_rl_repo")

import ml_dtypes

bf16 = ml_dtypes.bfloat16

N_CLS = 80
nA = 3
STRIDE = 8
IGNORE_THRE = 0.6
EPS = 1e-16
B = 16
K = 50
nG = 64
N_CORES = 8
P = 128
NCELL = nG * nG
f32 = np.float32

# tunables
DUP = 2          # duplicate gt scalars pairwise (bf16 2x packing aid)
DIL = 0.72       # band dilation factor (theory: (1/tau'-1) ~ 0.692 w/ bf16)
GPSIMD_TREE = False  # Pool engine rejects TENSOR_TENSOR (ISA check)
# batched-op sizes (slots per scan op); per-batch ns cost for the DP.
# G=32 measured 7us SLOWER than [16,16] despite fewer fixed overheads: a
# single batch loses batch-vs-tree pipelining and gates on the full g5 DMA.
G_COST = {16: 10600, 8: 5480, 4: 4510}


def _decompose(R):
    """Split R rounds into batch widths from G_COST minimizing total cost."""
    if R <= 0:
        return []
    best = {0: (0, ())}
    for s in range(1, R + 16):
        cands = []
        for g, c in G_COST.items():
            if s - g >= 0 and (s - g) in best:
                pc, pl = best[s - g]
                cands.append((pc + c, pl + (g,)))
        if cands:
            best[s] = min(cands)
    return list(min(best[s] for s in best if s >= R)[1])


# ---------------------------------------------------------------------------
# host-side label math (replicates reference.py semantics in f32 numpy)
# ---------------------------------------------------------------------------

def _host_precompute(labels, anchors_all, img_size):
    labels = np.asarray(labels, f32)
    anchors_all = np.asarray(anchors_all, f32)
    img_size = f32(img_size)
    anchors = anchors_all[:nA]
    norm_anch = anchors_all / img_size
    anch_w_n = anchors[:, 0] / img_size

    per_img = []
    for bb in range(B):
        lab = labels[bb]
        valid_row = lab.sum(-1) > 0
        tw, th = lab[:, 3], lab[:, 4]
        inter = np.minimum(tw[:, None], norm_anch[:, 0]) * np.minimum(
            th[:, None], norm_anch[:, 1]
        )
        union = tw[:, None] * th[:, None] + norm_anch[:, 0] * norm_anch[:, 1] - inter
        an_iou = inter / (union + f32(EPS))
        best_n_all = np.argmax(an_iou, axis=-1)
        best_n = best_n_all % nA
        valid = valid_row & (best_n_all < nA)

        ks = np.where(valid_row)[0]
        gcx, gcy, gw, gh = lab[ks, 1], lab[ks, 2], lab[ks, 3], lab[ks, 4]
        gt = dict(
            tlx=(gcx - gw / 2).astype(f32),
            tly=(gcy - gh / 2).astype(f32),
            brx=(gcx + gw / 2).astype(f32),
            bry=(gcy + gh / 2).astype(f32),
            area=(gw * gh).astype(f32),
            gh=gh.astype(f32),
        )

        tx = lab[:, 1] * nG
        ty = lab[:, 2] * nG
        ti = tx.astype(np.int32)
        tj = ty.astype(np.int32)
        tcls = lab[:, 0].astype(np.int32)
        lw, lh = lab[:, 3] * nG, lab[:, 4] * nG
        xc = np.floor(tx) + f32(0.5)
        yc = np.floor(ty) + f32(0.5)
        lab_ltrb = (
            np.maximum(
                np.stack(
                    [xc - (tx - lw / 2), yc - (ty - lh / 2),
                     (tx + lw / 2) - xc, (ty + lh / 2) - yc], -1),
                0.0,
            ) / f32(nG)
        ).astype(f32)
        cellmap = {}
        for k in range(K):
            if not valid[k]:
                continue
            key = (int(best_n[k]), int(tj[k]), int(ti[k]))
            tgt = np.log(lab_ltrb[k] / anch_w_n[best_n[k]] + f32(EPS)).astype(f32)
            if key not in cellmap:
                cellmap[key] = dict(tgt=tgt, cls=set([int(tcls[k])]))
            else:
                cellmap[key]["tgt"] = tgt  # scatter last-wins
                cellmap[key]["cls"].add(int(tcls[k]))

        # row-banded partition sets for the iou scan.  IoU > 0.6 forces the
        # cell center inside the gt box dilated by (2/3)*(wg,hg); partition
        # p holds row p//2, half p%2 (i<32 on even p, i>=32 on odd p).
        gt_n = len(ks)
        gw64 = (gt["brx"] - gt["tlx"]).astype(np.float64)
        gh64 = gt["gh"].astype(np.float64)
        ylo = gt["tly"].astype(np.float64) - DIL * gh64
        yhi = gt["bry"].astype(np.float64) + DIL * gh64
        xlo = gt["tlx"].astype(np.float64) - DIL * gw64
        xhi = gt["brx"].astype(np.float64) + DIL * gw64
        # extra pad row only for small boxes, where the DIL slack over the
        # theoretical 0.692 factor is below the bf16 coordinate noise
        rp_y = (gh64 < 0.1).astype(int)
        rp_x = (gw64 < 0.1).astype(int)
        j0 = np.clip(np.floor(ylo * nG - 0.5).astype(int) - rp_y, 0, nG - 1)
        j1 = np.clip(np.ceil(yhi * nG - 0.5).astype(int) + rp_y, 0, nG - 1)
        i0 = np.clip(np.floor(xlo * nG - 0.5).astype(int) - rp_x, 0, nG - 1)
        i1 = np.clip(np.ceil(xhi * nG - 0.5).astype(int) + rp_x, 0, nG - 1)
        # greedy set-packing into rounds (128-bit occupancy masks).
        # Images with no in-layer gt keep conf_loss_mask all-True in the
        # reference; skip their schedule so ACC stays at -100 -> ~ign = 1.
        sched = []  # (k, round, p0, p1, step)
        occ = []
        for k2 in sorted(range(gt_n if valid.any() else 0),
                         key=lambda q: (j0[q] - j1[q], q)):
            lo, hi = 2 * int(j0[k2]), 2 * int(j1[k2]) + 2
            if i1[k2] < 32:
                lo, step = lo, 2          # even partitions only
            elif i0[k2] >= 32:
                lo, step = lo + 1, 2      # odd partitions only
            else:
                step = 1
            mask = 0
            for p in range(lo, hi, step):
                mask |= 1 << p
            for r, o in enumerate(occ):
                if not (o & mask):
                    occ[r] |= mask
                    sched.append((k2, r, lo, hi, step))
                    break
            else:
                occ.append(mask)
                sched.append((k2, len(occ) - 1, lo, hi, step))
        per_img.append(dict(K=gt_n, gt=gt, cellmap=cellmap,
                            has_valid=bool(valid.any()),
                            sched=sched, R=len(occ)))
    return per_img


def _plan(labels, anchors_all, img_size):
    per_img = _host_precompute(labels, anchors_all, img_size)
    Rs = [info["R"] for info in per_img]
    order = sorted(range(B), key=lambda i: -Rs[i])
    A_imgs = order[:N_CORES]
    B_imgs = order[N_CORES:][::-1]  # pair big-A with small-B
    RA = max((Rs[i] for i in A_imgs), default=0)
    RB = max((Rs[i] for i in B_imgs), default=0)
    GA = tuple(_decompose(max(RA, 1)))
    if GA == (16, 16):
        # same slot count, but the smaller final batch shortens the exposed
        # tail tree on the critical path after the last wide op
        GA = (16, 8, 8)
    GB = tuple(_decompose(RB))  # empty when no B image has in-layer gts
    NGmax = 1
    for c in range(N_CORES):
        n = (len(per_img[A_imgs[c]]["cellmap"])
             + len(per_img[B_imgs[c]]["cellmap"]))
        NGmax = max(NGmax, n)
    NGmax = min(-(-NGmax // 8) * 8, P)
    return per_img, A_imgs, B_imgs, GA, GB, NGmax


# ---------------------------------------------------------------------------
# per-core input packing
# ---------------------------------------------------------------------------

def _pack_core_inputs(core, per_img, A_imgs, B_imgs, raw, img_size,
                      GA, GB, NGmax):
    img_size = f32(img_size)
    thre = f32(IGNORE_THRE)
    cthre = (thre / (f32(1.0) + thre)).astype(f32)
    imgs = [A_imgs[core], B_imgs[core]]
    SA, SB = sum(GA), sum(GB)

    # full-channel raw (gather source) + pre-packed 5-channel bf16 block
    rawsh = np.ascontiguousarray(raw[imgs]).reshape(2, 255, NCELL)
    # raw5 [P, 960]: col = im*480 + ch*96 + a*32 + c ; cell q = 32p + c
    r6 = rawsh.reshape(2, nA, 85, P, 32)[:, :, 0:5]       # im,a,ch,p,c
    raw5 = np.ascontiguousarray(
        r6.transpose(3, 0, 2, 1, 4).reshape(P, 960)).astype(bf16)

    # scan consts (bf16): per-(image, batch) g5 blocks + xyc
    # block for batch of g slots: col = comp*(g*DUP) + slot*DUP + dup,
    # per-partition values from the round schedule.  Per-batch blocks are
    # contiguous so each batch's first op waits only on its own DMA.
    blocks = []
    for glist, im in zip((GA, GB), imgs):
        info = per_img[im]
        sl = sum(glist)
        g5 = np.zeros((5, sl, P, DUP), f32)
        g5[4] = 1.0  # pad: prod - 1 <= 0 always (boxes within [0,1])
        gt = info["gt"]
        for k, r, p0, p1, step in info["sched"]:
            sl_ = slice(p0, p1, step)
            g5[0, r, sl_] = gt["tlx"][k]
            g5[1, r, sl_] = gt["tly"][k]
            g5[2, r, sl_] = gt["brx"][k]
            g5[3, r, sl_] = gt["bry"][k]
            g5[4, r, sl_] = cthre * (gt["area"][k] + f32(EPS))
        s0 = 0
        for g in glist:
            blk = g5[:, s0:s0 + g]  # (5, g, P, DUP)
            blocks.append(blk.transpose(2, 0, 1, 3).reshape(P, 5 * g * DUP))
            s0 += g

    # xyc [P,192]: col = comp*96 + aq, cell q = 32p + (aq % 32)
    pidx = np.arange(P)[:, None]
    aqidx = np.arange(96)[None, :]
    q = 32 * pidx + (aqidx % 32)
    gx = (q % nG).astype(f32)
    gy = (q // nG).astype(f32)
    xyc = np.concatenate([(gx + f32(0.5)) / f32(nG), (gy + f32(0.5)) / f32(nG)],
                         axis=1).astype(f32)
    consts_bf = np.ascontiguousarray(
        np.concatenate(blocks + [xyc], axis=1)).astype(bf16)

    # tail consts: ngm = 1 - gtmask (bf16); f32: tgt85, onehot, validng, gidx
    gtmask = np.zeros((P, 192), f32)
    cells = []
    for iml, im in enumerate(imgs):
        info = per_img[im]
        for (a, j, i), d in info["cellmap"].items():
            cq = j * nG + i
            gtmask[cq // 32, iml * 96 + a * 32 + cq % 32] = 1.0
            cells.append((iml, a, cq, d["tgt"], d["cls"]))
    ngm = np.ascontiguousarray(1.0 - gtmask).astype(bf16)

    tgt85 = np.zeros((P, 85), f32)
    onehot = np.zeros((P, 85), f32)
    validng = np.zeros((P, 85), f32)
    gidx = np.zeros((P, NGmax), np.int32)
    for g, (iml, a, cq, tgt, clsset) in enumerate(cells):
        tgt85[g, 0:4] = tgt
        onehot[g, 4] = 1.0
        for c in clsset:
            onehot[g, 5 + c] = 1.0
        validng[g, :] = 1.0
        gidx[:, g] = (iml * 255 + a * 85) * NCELL + cq
    consts_f = np.ascontiguousarray(np.concatenate(
        [tgt85, onehot, validng], axis=1)).astype(f32)
    return dict(rawsh=rawsh, raw5=raw5, cbf=consts_bf, cf=consts_f, ngm=ngm,
                gidx=np.ascontiguousarray(gidx))


# ---------------------------------------------------------------------------
# device program
# ---------------------------------------------------------------------------

def _build_program(GA, GB, NGmax, anchors_all, img_size):
    import concourse.bass as bass
    import concourse.mybir as mybir
    from concourse.tile import TileContext

    dtb = mybir.dt.bfloat16
    dtf = mybir.dt.float32
    AF = mybir.ActivationFunctionType
    OP = mybir.AluOpType
    AX = mybir.AxisListType
    cthre = float(IGNORE_THRE / (1.0 + IGNORE_THRE))
    SA, SB = sum(GA), sum(GB)
    W = max(GA + GB) * 96  # widest batch; tiles are sized for it

    nc = bass.Bass()

    lnaw_vals = [float(math.log(anchors_all[a][0] / img_size)) for a in range(nA)]

    rawsh = nc.declare_dram_parameter("rawsh", [2, 255, NCELL], dtf, False)
    raw5d = nc.declare_dram_parameter("raw5", [P, 960], dtb, False)
    CWS = 5 * (SA + SB) * DUP + 192
    blk_offs = []
    cur = 0
    for g in GA + GB:
        blk_offs.append(cur)
        cur += 5 * g * DUP
    off_xyc = cur
    cbf = nc.declare_dram_parameter("cbf", [P, CWS], dtb, False)
    CWF = 85 * 3
    off_tgt, off_oh, off_vn = 0, 85, 170
    cf = nc.declare_dram_parameter("cf", [P, CWF], dtf, False)
    gidxd = nc.declare_dram_parameter("gidx", [P, NGmax], mybir.dt.int32, False)
    ngmd = nc.declare_dram_parameter("ngm", [P, 192], dtb, False)
    out = nc.declare_dram_parameter("out", [P, 4], dtf, True)

    def A(t, offset, dims):
        h = t.tensor if hasattr(t, "tensor") else t
        return bass.AP(h, offset, dims)

    with TileContext(nc) as tc, \
            tc.tile_pool(name="main", bufs=1) as pool:
        RAW = pool.tile([P, 960], dtb, name="RAW")
        CS = pool.tile([P, CWS], dtb, name="CS")
        CF = pool.tile([P, CWF], dtf, name="CF")
        E = pool.tile([P, 768], dtb, name="E")
        TL = pool.tile([P, 384], dtb, name="TL")
        BR = pool.tile([P, 384], dtb, name="BR")
        WH = pool.tile([P, 384], dtb, name="WH")
        AREA = pool.tile([P, 192], dtb, name="AREA")
        ATH = pool.tile([P, 192], dtb, name="ATH")
        LNAW = pool.tile([P, 4], dtf, name="LNAW")
        IX = pool.tile([P, W], dtb, name="IX")
        AXT = pool.tile([P, W], dtb, name="AXT")
        IY = pool.tile([P, W], dtb, name="IY")
        AY = pool.tile([P, W], dtb, name="AY")
        IW = pool.tile([P, W], dtb, name="IW")
        IH = pool.tile([P, W], dtb, name="IH")
        PROD = pool.tile([P, W], dtb, name="PROD")
        T = [pool.tile([P, W], dtb, name=f"T{i}") for i in range(2)]
        TR1 = pool.tile([P, W // 2], dtb, name="TR1")
        TR2 = pool.tile([P, W // 4], dtb, name="TR2")
        ACC = pool.tile([P, 192], dtb, name="ACC")
        NGM = pool.tile([P, 192], dtb, name="NGM")
        MASK = pool.tile([P, 192], dtb, name="MASK")
        MEXCL = pool.tile([P, 192], dtf, name="MEXCL")
        SP = pool.tile([P, 192], dtf, name="SP")
        SPA = pool.tile([P, 192], dtf, name="SPA")
        SPB = pool.tile([P, 192], dtf, name="SPB")
        SPM = pool.tile([P, 192], dtf, name="SPM")
        GT85 = pool.tile([P, 85], dtf, name="GT85")
        U = pool.tile([P, 85], dtf, name="U")
        SPC = pool.tile([P, 85], dtf, name="SPC")
        OC = pool.tile([P, 85], dtf, name="OC")
        SPD = pool.tile([P, 85], dtf, name="SPD")
        OUTS = pool.tile([P, 4], dtf, name="OUTS")

        # ---- input loads.  The sync (SP) DGE queue is by far the fastest;
        # put the critical-path loads there in need-order.  CF/NGM (tail
        # consumers) trickle in on the slower gpsimd queue.  raw5 is split
        # per image so image A's pred prep starts ~2us earlier.
        # image A ltrb only (conf channels follow later; exps unblock sooner)
        nc.sync.dma_start(out=RAW[:, 0:384],
                          in_=A(raw5d, 0, [[960, P], [1, 384]]))
        nc.sync.dma_start(out=CS[:, off_xyc:off_xyc + 192],
                          in_=A(cbf, off_xyc, [[CWS, P], [1, 192]]))
        b0_end = blk_offs[1] if len(blk_offs) > 1 else off_xyc
        nc.sync.dma_start(out=CS[:, 0:b0_end],
                          in_=A(cbf, 0, [[CWS, P], [1, b0_end]]))
        if b0_end < off_xyc:
            nc.sync.dma_start(out=CS[:, b0_end:off_xyc],
                              in_=A(cbf, b0_end, [[CWS, P], [1, off_xyc - b0_end]]))
        if GB:
            nc.sync.dma_start(out=RAW[:, 480:864],
                              in_=A(raw5d, 480, [[960, P], [1, 384]]))
        # conf channels of both images in one strided DMA
        nc.sync.dma_start(out=A(RAW, 384, [[960, P], [480, 2], [1, 96]]),
                          in_=A(raw5d, 384, [[960, P], [480, 2], [1, 96]]))
        GIDX = pool.tile([P, NGmax], mybir.dt.int32, name="GIDX")
        nc.gpsimd.dma_start(out=GIDX[:], in_=gidxd[:])
        for a in range(nA):
            nc.gpsimd.memset(LNAW[:, a:a + 1], lnaw_vals[a])
        # dummy activation: forces the ACT table load to run before the
        # raw-data DMAs complete instead of stalling the first real exp
        nc.scalar.activation(out=LNAW[:, 3:4], in_=LNAW[:, 0:1], func=AF.Exp)

        # ---- sparse gt-cell gather (gpsimd queue, before batch trees; its
        # consumers are emitted after the scan so they don't stall DVE/ACT) ----
        nc.vector.memset(OUTS[:], 0.0)
        nc.gpsimd.indirect_dma_start(
            out=GT85[0:NGmax, 0:85],
            out_offset=None,
            in_=A(rawsh, 0, [[1, (2 * 255 - 85 + 1) * NCELL], [NCELL, 85], [1, 1]]),
            in_offset=bass.IndirectOffsetOnAxis(
                ap=A(GIDX, 0, [[NGmax, 1], [1, NGmax]]), axis=0),
        )
        # bulky tail consts load after the gather is in flight
        nc.gpsimd.dma_start(out=CF[:], in_=cf[:])
        nc.gpsimd.dma_start(out=NGM[:], in_=ngmd[:])

        # ---- pred prep (per image, so image A's scan starts before image
        # B's raw half lands) + banded iou scan ----
        nc.vector.memset(ACC[:], -100.0)
        treng = nc.gpsimd if GPSIMD_TREE else nc.vector
        xyc_b = A(CS, off_xyc, [[CWS, P], [96, 2], [1, 96]])

        def gt_ap(blk, comp, n):
            base = blk + comp * (n * DUP)
            return A(CS, base, [[CWS, P], [DUP, n], [0, 96 // DUP], [1, DUP]])

        def pred_ap(t, comp, im, n):
            return A(t, comp * 192 + im * 96, [[384, P], [0, n], [1, 96]])

        for im, glist in ((0, GA), (1, GB)):
            if not glist:
                # no in-layer gts on this slot: mask stays all-pass
                nc.vector.memset(MASK[:, im * 96:(im + 1) * 96], 1.0)
                continue
            for a in range(3):
                nc.scalar.activation(
                    out=A(E, im * 384 + a * 32, [[768, P], [96, 4], [1, 32]]),
                    in_=A(RAW, im * 480 + a * 32, [[960, P], [96, 4], [1, 32]]),
                    func=AF.Exp,
                    bias=A(LNAW, a, [[4, P], [1, 1]]),
                )
            e_lt = A(E, im * 384, [[768, P], [96, 2], [1, 96]])
            e_rb = A(E, im * 384 + 192, [[768, P], [96, 2], [1, 96]])
            quad = [[384, P], [192, 2], [1, 96]]
            nc.vector.tensor_tensor(out=A(TL, im * 96, quad), in0=xyc_b,
                                    in1=e_lt, op=OP.subtract)
            nc.vector.tensor_tensor(out=A(BR, im * 96, quad), in0=xyc_b,
                                    in1=e_rb, op=OP.add)
            nc.vector.tensor_tensor(out=A(WH, im * 96, quad), in0=e_lt,
                                    in1=e_rb, op=OP.add)
            sl96 = slice(im * 96, im * 96 + 96)
            nc.vector.tensor_tensor(out=AREA[:, sl96], in0=WH[:, sl96],
                                    in1=WH[:, 192 + im * 96:192 + im * 96 + 96],
                                    op=OP.mult)
            nc.vector.tensor_scalar(out=ATH[:, sl96], in0=AREA[:, sl96],
                                    scalar1=cthre, scalar2=None, op0=OP.mult)
            for bi, g in enumerate(glist):
                wb = g * 96
                blk = blk_offs[(0 if im == 0 else len(GA)) + bi]
                tt = T[bi % 2]
                nc.vector.tensor_tensor(out=IX[:, 0:wb],
                                        in0=pred_ap(TL, 0, im, g),
                                        in1=gt_ap(blk, 0, g), op=OP.max)
                nc.vector.tensor_tensor(out=AXT[:, 0:wb],
                                        in0=pred_ap(BR, 0, im, g),
                                        in1=gt_ap(blk, 2, g), op=OP.min)
                nc.vector.tensor_tensor(out=IW[:, 0:wb], in0=AXT[:, 0:wb],
                                        in1=IX[:, 0:wb], op=OP.subtract)
                nc.vector.tensor_tensor(out=IY[:, 0:wb],
                                        in0=pred_ap(TL, 1, im, g),
                                        in1=gt_ap(blk, 1, g), op=OP.max)
                nc.vector.tensor_tensor(out=AY[:, 0:wb],
                                        in0=pred_ap(BR, 1, im, g),
                                        in1=gt_ap(blk, 3, g), op=OP.min)
                nc.vector.tensor_tensor(out=IH[:, 0:wb], in0=AY[:, 0:wb],
                                        in1=IY[:, 0:wb], op=OP.subtract)
                # clamp ih only: iw<0 or ih<0 both give prod <= 0 < ath
                nc.vector.tensor_scalar(out=IH[:, 0:wb], in0=IH[:, 0:wb],
                                        scalar1=0.0, scalar2=None, op0=OP.max)
                nc.vector.tensor_tensor(out=PROD[:, 0:wb], in0=IW[:, 0:wb],
                                        in1=IH[:, 0:wb], op=OP.mult)
                nc.vector.tensor_tensor(out=tt[:, 0:wb], in0=PROD[:, 0:wb],
                                        in1=gt_ap(blk, 4, g),
                                        op=OP.subtract)
                # tree-max margins over rounds -> ACC
                width, src, flip = wb, tt, 0
                while width > 96:
                    h = width // 2
                    dst = (TR1, TR2)[flip % 2]
                    treng.tensor_tensor(out=dst[:, 0:h], in0=src[:, 0:h],
                                        in1=src[:, h:2 * h], op=OP.max)
                    src, width, flip = dst, h, flip + 1
                acc_sl = ACC[:, im * 96:(im + 1) * 96]
                treng.tensor_tensor(out=acc_sl, in0=acc_sl, in1=src[:, 0:96],
                                    op=OP.max)
            # notign = (max margin <= athp) for this image
            nc.vector.tensor_tensor(out=MASK[:, im * 96:(im + 1) * 96],
                                    in0=ACC[:, im * 96:(im + 1) * 96],
                                    in1=ATH[:, im * 96:(im + 1) * 96],
                                    op=OP.is_le)

        # conf softplus on ACT while DVE scans
        conf_view = A(RAW, 384, [[960, P], [480, 2], [1, 96]])
        sp_flat = [[192, P], [96, 2], [1, 96]]
        nc.scalar.activation(out=A(SPA, 0, sp_flat), in_=conf_view, func=AF.Abs)
        nc.scalar.activation(out=SPB[:], in_=SPA[:], func=AF.Exp, scale=-1.0)
        nc.scalar.activation(out=SPA[:], in_=SPB[:], func=AF.Ln, bias=1.0)
        nc.scalar.activation(out=A(SPB, 0, sp_flat), in_=conf_view, func=AF.Relu)

        # ---- sparse gt-cell terms (consumers of the early gather) ----
        nc.scalar.activation(out=SPC[0:NGmax, 4:85], in_=GT85[0:NGmax, 4:85],
                             func=AF.Abs)
        nc.scalar.activation(out=SPD[0:NGmax, 4:85], in_=SPC[0:NGmax, 4:85],
                             func=AF.Exp, scale=-1.0)
        nc.scalar.activation(out=SPC[0:NGmax, 4:85], in_=SPD[0:NGmax, 4:85],
                             func=AF.Ln, bias=1.0)
        nc.scalar.activation(out=SPD[0:NGmax, 4:85], in_=GT85[0:NGmax, 4:85],
                             func=AF.Relu)
        nc.vector.tensor_tensor(out=OC[0:NGmax, 4:85], in0=GT85[0:NGmax, 4:85],
                                in1=A(CF, off_oh + 4, [[CWF, NGmax], [1, 81]]),
                                op=OP.mult)
        nc.vector.tensor_tensor(out=U[0:NGmax, 4:85], in0=SPC[0:NGmax, 4:85],
                                in1=SPD[0:NGmax, 4:85], op=OP.add)
        nc.vector.tensor_tensor(out=U[0:NGmax, 4:85], in0=U[0:NGmax, 4:85],
                                in1=OC[0:NGmax, 4:85], op=OP.subtract)
        nc.vector.tensor_tensor(out=OC[0:NGmax, 0:4], in0=GT85[0:NGmax, 0:4],
                                in1=A(CF, off_tgt, [[CWF, NGmax], [1, 4]]),
                                op=OP.subtract)
        nc.scalar.activation(out=U[0:NGmax, 0:4], in_=OC[0:NGmax, 0:4],
                             func=AF.Square)
        # ---- tail: only ops that depend on the scan or the bbox Square ----
        nc.vector.tensor_tensor(out=U[0:NGmax, :], in0=U[0:NGmax, :],
                                in1=A(CF, off_vn, [[CWF, NGmax], [1, 85]]),
                                op=OP.mult)
        nc.vector.reduce_sum(out=OUTS[0:NGmax, 2:3], in_=U[0:NGmax, :],
                             axis=AX.X)
        nc.vector.tensor_tensor(out=SP[:], in0=SPA[:], in1=SPB[:], op=OP.add)
        nc.vector.tensor_tensor(out=MEXCL[:], in0=SP[:], in1=NGM[:],
                                op=OP.mult)
        for im in (1, 0):
            sl96 = slice(im * 96, im * 96 + 96)
            nc.vector.tensor_tensor(out=SPM[:, sl96], in0=MEXCL[:, sl96],
                                    in1=MASK[:, sl96], op=OP.mult)
            nc.vector.reduce_sum(out=OUTS[:, im:im + 1], in_=SPM[:, sl96],
                                 axis=AX.X)

        nc.sync.dma_start(out=out[:], in_=OUTS[:])

    return nc


_CACHE = {}
TRACE = False
LAST_RESULTS = None


def _split_multiwait(nc):
    """Walrus codegen on this toolchain supports only one sync-wait command
    per instruction; split multi-wait instructions into single-wait NOPs on
    the same engine."""
    import concourse.mybir as mybir

    if getattr(nc, "_fcos_wait_split", False):
        return
    nc._fcos_wait_split = True
    for bb in nc.m.functions[0].blocks:
        insts = bb.instructions
        for ins in list(insts):
            si = ins.sync_info
            if si is not None and len(si.on_wait) > 1:
                waits = list(si.on_wait)
                idx = insts.index(ins)
                nops = []
                for j, w in enumerate(waits[:-1]):
                    nop = mybir.InstNoOp(name=f"{ins.name}-wsplit{j}", ins=[],
                                         outs=[])
                    nop.engine = ins.engine
                    nop.sync_info = mybir.SyncInfo(on_wait=[w], on_update=[])
                    nops.append(nop)
                ins.sync_info = mybir.SyncInfo(on_wait=[waits[-1]],
                                               on_update=list(si.on_update))
                for nop in reversed(nops):
                    insts.insert(idx, nop)


def kernel(raw, labels, anchors_all, img_size):
    from concourse.bass_utils import run_bass_kernel_spmd

    raw = np.asarray(raw, f32)
    labels_np = np.asarray(labels, f32)
    anchors_np = np.asarray(anchors_all, f32)
    isize = int(img_size)

    per_img, A_imgs, B_imgs, GA, GB, NGmax = _plan(labels_np, anchors_np, isize)
    key = (GA, GB, NGmax, DUP, GPSIMD_TREE, anchors_np.tobytes(), isize)
    if key not in _CACHE:
        _CACHE[key] = _build_program(GA, GB, NGmax, anchors_np.tolist(), isize)
    nc = _CACHE[key]
    _split_multiwait(nc)

    in_maps = [
        _pack_core_inputs(c, per_img, A_imgs, B_imgs, raw, isize, GA, GB, NGmax)
        for c in range(N_CORES)
    ]
    global LAST_RESULTS
    res = run_bass_kernel_spmd(nc, in_maps, list(range(N_CORES)), trace=TRACE)
    LAST_RESULTS = res
    total = np.float64(0.0)
    for c in range(N_CORES):
        o = res.results[c]["out"]
        total += np.sum(o[:, 0:3], dtype=np.float64)
    return f32(total)


if __name__ == "__main__":
    import importlib.util

    spec = importlib.util.spec_from_file_location("reference",
                                                  "/root/problem/reference.py")
    ref = importlib.util.module_from_spec(spec)
    spec.loader.exec_module(ref)
    inputs = ref.setup_inputs()
    np_inputs = {k: np.asarray(v) for k, v in inputs.items()}
    got = kernel(**np_inputs)
    print("kernel:", got)


# revision 69
# speedup vs baseline: 1.0394x; 1.0394x over previous
"""Trainium2 Bass kernel for the YOLO/FCOS-layer loss (nn_FCOSLayer_22840636080477).

Sharding: data-parallel over batch, 2 images per NeuronCore x 8 cores, one
SPMD program. Host does label-side preprocessing (anchor matching, scatter
dedup, row-band gt->partition scheduling, constant packing); device does
everything that touches `raw`:

  loss = sum_cells softplus(conf) * (conf_mask & ~gt)          (dense)
       + sum_gtcells [ softplus(conf)-conf                      (sparse, gather)
                      + sum_c (softplus(cls_c) - onehot_c*cls_c)
                      + sum_4 (ltrb_raw - tgt)^2 ]

The ignore mask needs a max-IoU scan of 12288 pred boxes against each
image's gt boxes.  Three structural tricks make it cheap:

1. bf16 + packed last-axis APs -> DVE 2x_1p mode (0.52 ns/elem).
2. Row banding: IoU > 0.6 forces the pred's cell center inside the gt
   box dilated by (2/3)*(wg,hg) (pred-independent bound), so each gt
   only concerns a contiguous band of partitions (partition p holds
   row p//2).  Interval-coloring packs gts into "rounds"; each round
   instruction gives every partition its own gt via per-partition
   consts.  Rounds ~ max-load instead of K.
3. Margin m = min(iw*ih - athg, iw) has the exact sign of the ignore
   condition (valid when gt heights <= 1), killing both clamps; the
   per-pred threshold athp = cthre*areap is compared once at the end:
   ignore <=> max_g m_g > athp.
"""
import sys
import math
import numpy as np

sys.path.insert(0, "/opt/trn_rl_repo")

import ml_dtypes

bf16 = ml_dtypes.bfloat16

N_CLS = 80
nA = 3
STRIDE = 8
IGNORE_THRE = 0.6
EPS = 1e-16
B = 16
K = 50
nG = 64
N_CORES = 8
P = 128
NCELL = nG * nG
f32 = np.float32

# tunables
DUP = 2          # duplicate gt scalars pairwise (bf16 2x packing aid)
DIL = 0.72       # band dilation factor (theory: (1/tau'-1) ~ 0.692 w/ bf16)
GPSIMD_TREE = False  # Pool engine rejects TENSOR_TENSOR (ISA check)
# batched-op sizes (slots per scan op); per-slot ns cost for the DP
G_COST = {16: 10600, 8: 5480, 4: 4510}


def _decompose(R):
    """Split R rounds into batch widths from G_COST minimizing total cost."""
    if R <= 0:
        return []
    best = {0: (0, ())}
    for s in range(1, R + 16):
        cands = []
        for g, c in G_COST.items():
            if s - g >= 0 and (s - g) in best:
                pc, pl = best[s - g]
                cands.append((pc + c, pl + (g,)))
        if cands:
            best[s] = min(cands)
    return list(min(best[s] for s in best if s >= R)[1])


# ---------------------------------------------------------------------------
# host-side label math (replicates reference.py semantics in f32 numpy)
# ---------------------------------------------------------------------------

def _host_precompute(labels, anchors_all, img_size):
    labels = np.asarray(labels, f32)
    anchors_all = np.asarray(anchors_all, f32)
    img_size = f32(img_size)
    anchors = anchors_all[:nA]
    norm_anch = anchors_all / img_size
    anch_w_n = anchors[:, 0] / img_size

    per_img = []
    for bb in range(B):
        lab = labels[bb]
        valid_row = lab.sum(-1) > 0
        tw, th = lab[:, 3], lab[:, 4]
        inter = np.minimum(tw[:, None], norm_anch[:, 0]) * np.minimum(
            th[:, None], norm_anch[:, 1]
        )
        union = tw[:, None] * th[:, None] + norm_anch[:, 0] * norm_anch[:, 1] - inter
        an_iou = inter / (union + f32(EPS))
        best_n_all = np.argmax(an_iou, axis=-1)
        best_n = best_n_all % nA
        valid = valid_row & (best_n_all < nA)

        ks = np.where(valid_row)[0]
        gcx, gcy, gw, gh = lab[ks, 1], lab[ks, 2], lab[ks, 3], lab[ks, 4]
        gt = dict(
            tlx=(gcx - gw / 2).astype(f32),
            tly=(gcy - gh / 2).astype(f32),
            brx=(gcx + gw / 2).astype(f32),
            bry=(gcy + gh / 2).astype(f32),
            area=(gw * gh).astype(f32),
            gh=gh.astype(f32),
        )

        tx = lab[:, 1] * nG
        ty = lab[:, 2] * nG
        ti = tx.astype(np.int32)
        tj = ty.astype(np.int32)
        tcls = lab[:, 0].astype(np.int32)
        lw, lh = lab[:, 3] * nG, lab[:, 4] * nG
        xc = np.floor(tx) + f32(0.5)
        yc = np.floor(ty) + f32(0.5)
        lab_ltrb = (
            np.maximum(
                np.stack(
                    [xc - (tx - lw / 2), yc - (ty - lh / 2),
                     (tx + lw / 2) - xc, (ty + lh / 2) - yc], -1),
                0.0,
            ) / f32(nG)
        ).astype(f32)
        cellmap = {}
        for k in range(K):
            if not valid[k]:
                continue
            key = (int(best_n[k]), int(tj[k]), int(ti[k]))
            tgt = np.log(lab_ltrb[k] / anch_w_n[best_n[k]] + f32(EPS)).astype(f32)
            if key not in cellmap:
                cellmap[key] = dict(tgt=tgt, cls=set([int(tcls[k])]))
            else:
                cellmap[key]["tgt"] = tgt  # scatter last-wins
                cellmap[key]["cls"].add(int(tcls[k]))

        # row-banded partition sets for the iou scan.  IoU > 0.6 forces the
        # cell center inside the gt box dilated by (2/3)*(wg,hg); partition
        # p holds row p//2, half p%2 (i<32 on even p, i>=32 on odd p).
        gt_n = len(ks)
        gw64 = (gt["brx"] - gt["tlx"]).astype(np.float64)
        gh64 = gt["gh"].astype(np.float64)
        ylo = gt["tly"].astype(np.float64) - DIL * gh64
        yhi = gt["bry"].astype(np.float64) + DIL * gh64
        xlo = gt["tlx"].astype(np.float64) - DIL * gw64
        xhi = gt["brx"].astype(np.float64) + DIL * gw64
        # extra pad row only for small boxes, where the DIL slack over the
        # theoretical 0.692 factor is below the bf16 coordinate noise
        rp_y = (gh64 < 0.1).astype(int)
        rp_x = (gw64 < 0.1).astype(int)
        j0 = np.clip(np.floor(ylo * nG - 0.5).astype(int) - rp_y, 0, nG - 1)
        j1 = np.clip(np.ceil(yhi * nG - 0.5).astype(int) + rp_y, 0, nG - 1)
        i0 = np.clip(np.floor(xlo * nG - 0.5).astype(int) - rp_x, 0, nG - 1)
        i1 = np.clip(np.ceil(xhi * nG - 0.5).astype(int) + rp_x, 0, nG - 1)
        # greedy set-packing into rounds (128-bit occupancy masks).
        # Images with no in-layer gt keep conf_loss_mask all-True in the
        # reference; skip their schedule so ACC stays at -100 -> ~ign = 1.
        sched = []  # (k, round, p0, p1, step)
        occ = []
        for k2 in sorted(range(gt_n if valid.any() else 0),
                         key=lambda q: (j0[q] - j1[q], q)):
            lo, hi = 2 * int(j0[k2]), 2 * int(j1[k2]) + 2
            if i1[k2] < 32:
                lo, step = lo, 2          # even partitions only
            elif i0[k2] >= 32:
                lo, step = lo + 1, 2      # odd partitions only
            else:
                step = 1
            mask = 0
            for p in range(lo, hi, step):
                mask |= 1 << p
            for r, o in enumerate(occ):
                if not (o & mask):
                    occ[r] |= mask
                    sched.append((k2, r, lo, hi, step))
                    break
            else:
                occ.append(mask)
                sched.append((k2, len(occ) - 1, lo, hi, step))
        per_img.append(dict(K=gt_n, gt=gt, cellmap=cellmap,
                            has_valid=bool(valid.any()),
                            sched=sched, R=len(occ)))
    return per_img


def _plan(labels, anchors_all, img_size):
    per_img = _host_precompute(labels, anchors_all, img_size)
    Rs = [info["R"] for info in per_img]
    order = sorted(range(B), key=lambda i: -Rs[i])
    A_imgs = order[:N_CORES]
    B_imgs = order[N_CORES:][::-1]  # pair big-A with small-B
    RA = max((Rs[i] for i in A_imgs), default=0)
    RB = max((Rs[i] for i in B_imgs), default=0)
    GA = tuple(_decompose(max(RA, 1)))
    GB = tuple(_decompose(RB))  # empty when no B image has in-layer gts
    NGmax = 1
    for c in range(N_CORES):
        n = (len(per_img[A_imgs[c]]["cellmap"])
             + len(per_img[B_imgs[c]]["cellmap"]))
        NGmax = max(NGmax, n)
    NGmax = min(-(-NGmax // 8) * 8, P)
    return per_img, A_imgs, B_imgs, GA, GB, NGmax


# ---------------------------------------------------------------------------
# per-core input packing
# ---------------------------------------------------------------------------

def _pack_core_inputs(core, per_img, A_imgs, B_imgs, raw, img_size,
                      GA, GB, NGmax):
    img_size = f32(img_size)
    thre = f32(IGNORE_THRE)
    cthre = (thre / (f32(1.0) + thre)).astype(f32)
    imgs = [A_imgs[core], B_imgs[core]]
    SA, SB = sum(GA), sum(GB)

    # full-channel raw (gather source) + pre-packed 5-channel bf16 block
    rawsh = np.ascontiguousarray(raw[imgs]).reshape(2, 255, NCELL)
    # raw5 [P, 960]: col = im*480 + ch*96 + a*32 + c ; cell q = 32p + c
    r6 = rawsh.reshape(2, nA, 85, P, 32)[:, :, 0:5]       # im,a,ch,p,c
    raw5 = np.ascontiguousarray(
        r6.transpose(3, 0, 2, 1, 4).reshape(P, 960)).astype(bf16)

    # scan consts (bf16): per-(image, batch) g5 blocks + xyc
    # block for batch of g slots: col = comp*(g*DUP) + slot*DUP + dup,
    # per-partition values from the round schedule.  Per-batch blocks are
    # contiguous so each batch's first op waits only on its own DMA.
    blocks = []
    for glist, im in zip((GA, GB), imgs):
        info = per_img[im]
        sl = sum(glist)
        g5 = np.zeros((5, sl, P, DUP), f32)
        g5[4] = 1.0  # pad: prod - 1 <= 0 always (boxes within [0,1])
        gt = info["gt"]
        for k, r, p0, p1, step in info["sched"]:
            sl_ = slice(p0, p1, step)
            g5[0, r, sl_] = gt["tlx"][k]
            g5[1, r, sl_] = gt["tly"][k]
            g5[2, r, sl_] = gt["brx"][k]
            g5[3, r, sl_] = gt["bry"][k]
            g5[4, r, sl_] = cthre * (gt["area"][k] + f32(EPS))
        s0 = 0
        for g in glist:
            blk = g5[:, s0:s0 + g]  # (5, g, P, DUP)
            blocks.append(blk.transpose(2, 0, 1, 3).reshape(P, 5 * g * DUP))
            s0 += g

    # xyc [P,192]: col = comp*96 + aq, cell q = 32p + (aq % 32)
    pidx = np.arange(P)[:, None]
    aqidx = np.arange(96)[None, :]
    q = 32 * pidx + (aqidx % 32)
    gx = (q % nG).astype(f32)
    gy = (q // nG).astype(f32)
    xyc = np.concatenate([(gx + f32(0.5)) / f32(nG), (gy + f32(0.5)) / f32(nG)],
                         axis=1).astype(f32)
    consts_bf = np.ascontiguousarray(
        np.concatenate(blocks + [xyc], axis=1)).astype(bf16)

    # tail consts: ngm = 1 - gtmask (bf16); f32: tgt85, onehot, validng, gidx
    gtmask = np.zeros((P, 192), f32)
    cells = []
    for iml, im in enumerate(imgs):
        info = per_img[im]
        for (a, j, i), d in info["cellmap"].items():
            cq = j * nG + i
            gtmask[cq // 32, iml * 96 + a * 32 + cq % 32] = 1.0
            cells.append((iml, a, cq, d["tgt"], d["cls"]))
    ngm = np.ascontiguousarray(1.0 - gtmask).astype(bf16)

    tgt85 = np.zeros((P, 85), f32)
    onehot = np.zeros((P, 85), f32)
    validng = np.zeros((P, 85), f32)
    gidx = np.zeros((P, NGmax), np.int32)
    for g, (iml, a, cq, tgt, clsset) in enumerate(cells):
        tgt85[g, 0:4] = tgt
        onehot[g, 4] = 1.0
        for c in clsset:
            onehot[g, 5 + c] = 1.0
        validng[g, :] = 1.0
        gidx[:, g] = (iml * 255 + a * 85) * NCELL + cq
    consts_f = np.ascontiguousarray(np.concatenate(
        [tgt85, onehot, validng], axis=1)).astype(f32)
    return dict(rawsh=rawsh, raw5=raw5, cbf=consts_bf, cf=consts_f, ngm=ngm,
                gidx=np.ascontiguousarray(gidx))


# ---------------------------------------------------------------------------
# device program
# ---------------------------------------------------------------------------

def _build_program(GA, GB, NGmax, anchors_all, img_size):
    import concourse.bass as bass
    import concourse.mybir as mybir
    from concourse.tile import TileContext

    dtb = mybir.dt.bfloat16
    dtf = mybir.dt.float32
    AF = mybir.ActivationFunctionType
    OP = mybir.AluOpType
    AX = mybir.AxisListType
    cthre = float(IGNORE_THRE / (1.0 + IGNORE_THRE))
    SA, SB = sum(GA), sum(GB)
    W = max(GA + GB) * 96  # widest batch; tiles are sized for it

    nc = bass.Bass()

    lnaw_vals = [float(math.log(anchors_all[a][0] / img_size)) for a in range(nA)]

    rawsh = nc.declare_dram_parameter("rawsh", [2, 255, NCELL], dtf, False)
    raw5d = nc.declare_dram_parameter("raw5", [P, 960], dtb, False)
    CWS = 5 * (SA + SB) * DUP + 192
    blk_offs = []
    cur = 0
    for g in GA + GB:
        blk_offs.append(cur)
        cur += 5 * g * DUP
    off_xyc = cur
    cbf = nc.declare_dram_parameter("cbf", [P, CWS], dtb, False)
    CWF = 85 * 3
    off_tgt, off_oh, off_vn = 0, 85, 170
    cf = nc.declare_dram_parameter("cf", [P, CWF], dtf, False)
    gidxd = nc.declare_dram_parameter("gidx", [P, NGmax], mybir.dt.int32, False)
    ngmd = nc.declare_dram_parameter("ngm", [P, 192], dtb, False)
    out = nc.declare_dram_parameter("out", [P, 4], dtf, True)

    def A(t, offset, dims):
        h = t.tensor if hasattr(t, "tensor") else t
        return bass.AP(h, offset, dims)

    with TileContext(nc) as tc, \
            tc.tile_pool(name="main", bufs=1) as pool:
        RAW = pool.tile([P, 960], dtb, name="RAW")
        CS = pool.tile([P, CWS], dtb, name="CS")
        CF = pool.tile([P, CWF], dtf, name="CF")
        E = pool.tile([P, 768], dtb, name="E")
        TL = pool.tile([P, 384], dtb, name="TL")
        BR = pool.tile([P, 384], dtb, name="BR")
        WH = pool.tile([P, 384], dtb, name="WH")
        AREA = pool.tile([P, 192], dtb, name="AREA")
        ATH = pool.tile([P, 192], dtb, name="ATH")
        LNAW = pool.tile([P, 4], dtf, name="LNAW")
        IX = pool.tile([P, W], dtb, name="IX")
        AXT = pool.tile([P, W], dtb, name="AXT")
        IY = pool.tile([P, W], dtb, name="IY")
        AY = pool.tile([P, W], dtb, name="AY")
        IW = pool.tile([P, W], dtb, name="IW")
        IH = [pool.tile([P, W], dtb, name=f"IH{i}") for i in range(2)]
        IHC = [pool.tile([P, W], dtb, name=f"IHC{i}") for i in range(2)]
        PROD = pool.tile([P, W], dtb, name="PROD")
        T = [pool.tile([P, W], dtb, name=f"T{i}") for i in range(2)]
        TR1 = pool.tile([P, W // 2], dtb, name="TR1")
        TR2 = pool.tile([P, W // 4], dtb, name="TR2")
        ACC = pool.tile([P, 192], dtb, name="ACC")
        NGM = pool.tile([P, 192], dtb, name="NGM")
        MASK = pool.tile([P, 192], dtb, name="MASK")
        MEXCL = pool.tile([P, 192], dtf, name="MEXCL")
        SP = pool.tile([P, 192], dtf, name="SP")
        SPA = pool.tile([P, 192], dtf, name="SPA")
        SPB = pool.tile([P, 192], dtf, name="SPB")
        SPM = pool.tile([P, 192], dtf, name="SPM")
        GT85 = pool.tile([P, 85], dtf, name="GT85")
        U = pool.tile([P, 85], dtf, name="U")
        SPC = pool.tile([P, 85], dtf, name="SPC")
        OC = pool.tile([P, 85], dtf, name="OC")
        SPD = pool.tile([P, 85], dtf, name="SPD")
        OUTS = pool.tile([P, 4], dtf, name="OUTS")

        # ---- input loads.  The sync (SP) DGE queue is by far the fastest;
        # put the critical-path loads there in need-order.  CF/NGM (tail
        # consumers) trickle in on the slower gpsimd queue.  raw5 is split
        # per image so image A's pred prep starts ~2us earlier.
        # image A ltrb only (conf channels follow later; exps unblock sooner)
        nc.sync.dma_start(out=RAW[:, 0:384],
                          in_=A(raw5d, 0, [[960, P], [1, 384]]))
        nc.sync.dma_start(out=CS[:, off_xyc:off_xyc + 192],
                          in_=A(cbf, off_xyc, [[CWS, P], [1, 192]]))
        b0_end = blk_offs[1] if len(blk_offs) > 1 else off_xyc
        nc.sync.dma_start(out=CS[:, 0:b0_end],
                          in_=A(cbf, 0, [[CWS, P], [1, b0_end]]))
        if b0_end < off_xyc:
            nc.sync.dma_start(out=CS[:, b0_end:off_xyc],
                              in_=A(cbf, b0_end, [[CWS, P], [1, off_xyc - b0_end]]))
        if GB:
            nc.sync.dma_start(out=RAW[:, 480:864],
                              in_=A(raw5d, 480, [[960, P], [1, 384]]))
        # conf channels of both images in one strided DMA
        nc.sync.dma_start(out=A(RAW, 384, [[960, P], [480, 2], [1, 96]]),
                          in_=A(raw5d, 384, [[960, P], [480, 2], [1, 96]]))
        GIDX = pool.tile([P, NGmax], mybir.dt.int32, name="GIDX")
        nc.gpsimd.dma_start(out=GIDX[:], in_=gidxd[:])
        for a in range(nA):
            nc.gpsimd.memset(LNAW[:, a:a + 1], lnaw_vals[a])
        # dummy activation: forces the ACT table load to run before the
        # raw-data DMAs complete instead of stalling the first real exp
        nc.scalar.activation(out=LNAW[:, 3:4], in_=LNAW[:, 0:1], func=AF.Exp)

        # ---- sparse gt-cell gather (gpsimd queue, before batch trees; its
        # consumers are emitted after the scan so they don't stall DVE/ACT) ----
        nc.vector.memset(OUTS[:], 0.0)
        nc.gpsimd.indirect_dma_start(
            out=GT85[0:NGmax, 0:85],
            out_offset=None,
            in_=A(rawsh, 0, [[1, (2 * 255 - 85 + 1) * NCELL], [NCELL, 85], [1, 1]]),
            in_offset=bass.IndirectOffsetOnAxis(
                ap=A(GIDX, 0, [[NGmax, 1], [1, NGmax]]), axis=0),
        )
        # bulky tail consts load after the gather is in flight
        nc.gpsimd.dma_start(out=CF[:], in_=cf[:])
        nc.gpsimd.dma_start(out=NGM[:], in_=ngmd[:])

        # ---- pred prep (per image, so image A's scan starts before image
        # B's raw half lands) + banded iou scan ----
        nc.vector.memset(ACC[:], -100.0)
        treng = nc.gpsimd if GPSIMD_TREE else nc.vector
        xyc_b = A(CS, off_xyc, [[CWS, P], [96, 2], [1, 96]])

        def gt_ap(blk, comp, n):
            base = blk + comp * (n * DUP)
            return A(CS, base, [[CWS, P], [DUP, n], [0, 96 // DUP], [1, DUP]])

        def pred_ap(t, comp, im, n):
            return A(t, comp * 192 + im * 96, [[384, P], [0, n], [1, 96]])

        for im, glist in ((0, GA), (1, GB)):
            if not glist:
                # no in-layer gts on this slot: mask stays all-pass
                nc.vector.memset(MASK[:, im * 96:(im + 1) * 96], 1.0)
                continue
            for a in range(3):
                nc.scalar.activation(
                    out=A(E, im * 384 + a * 32, [[768, P], [96, 4], [1, 32]]),
                    in_=A(RAW, im * 480 + a * 32, [[960, P], [96, 4], [1, 32]]),
                    func=AF.Exp,
                    bias=A(LNAW, a, [[4, P], [1, 1]]),
                )
            e_lt = A(E, im * 384, [[768, P], [96, 2], [1, 96]])
            e_rb = A(E, im * 384 + 192, [[768, P], [96, 2], [1, 96]])
            quad = [[384, P], [192, 2], [1, 96]]
            nc.vector.tensor_tensor(out=A(TL, im * 96, quad), in0=xyc_b,
                                    in1=e_lt, op=OP.subtract)
            nc.vector.tensor_tensor(out=A(BR, im * 96, quad), in0=xyc_b,
                                    in1=e_rb, op=OP.add)
            nc.vector.tensor_tensor(out=A(WH, im * 96, quad), in0=e_lt,
                                    in1=e_rb, op=OP.add)
            sl96 = slice(im * 96, im * 96 + 96)
            nc.vector.tensor_tensor(out=AREA[:, sl96], in0=WH[:, sl96],
                                    in1=WH[:, 192 + im * 96:192 + im * 96 + 96],
                                    op=OP.mult)
            nc.vector.tensor_scalar(out=ATH[:, sl96], in0=AREA[:, sl96],
                                    scalar1=cthre, scalar2=None, op0=OP.mult)
            for bi, g in enumerate(glist):
                wb = g * 96
                blk = blk_offs[(0 if im == 0 else len(GA)) + bi]
                tt = T[bi % 2]
                ihb, ihcb = IH[bi % 2], IHC[bi % 2]
                # y-chain first: the ih clamp runs on the idle ACT engine,
                # hidden under the x-chain's DVE work
                nc.vector.tensor_tensor(out=IY[:, 0:wb],
                                        in0=pred_ap(TL, 1, im, g),
                                        in1=gt_ap(blk, 1, g), op=OP.max)
                nc.vector.tensor_tensor(out=AY[:, 0:wb],
                                        in0=pred_ap(BR, 1, im, g),
                                        in1=gt_ap(blk, 3, g), op=OP.min)
                nc.vector.tensor_tensor(out=ihb[:, 0:wb], in0=AY[:, 0:wb],
                                        in1=IY[:, 0:wb], op=OP.subtract)
                # clamp ih only: iw<0 or ih<0 both give prod <= 0 < ath
                nc.scalar.activation(out=ihcb[:, 0:wb], in_=ihb[:, 0:wb],
                                     func=AF.Relu)
                nc.vector.tensor_tensor(out=IX[:, 0:wb],
                                        in0=pred_ap(TL, 0, im, g),
                                        in1=gt_ap(blk, 0, g), op=OP.max)
                nc.vector.tensor_tensor(out=AXT[:, 0:wb],
                                        in0=pred_ap(BR, 0, im, g),
                                        in1=gt_ap(blk, 2, g), op=OP.min)
                nc.vector.tensor_tensor(out=IW[:, 0:wb], in0=AXT[:, 0:wb],
                                        in1=IX[:, 0:wb], op=OP.subtract)
                nc.vector.tensor_tensor(out=PROD[:, 0:wb], in0=IW[:, 0:wb],
                                        in1=ihcb[:, 0:wb], op=OP.mult)
                nc.vector.tensor_tensor(out=tt[:, 0:wb], in0=PROD[:, 0:wb],
                                        in1=gt_ap(blk, 4, g),
                                        op=OP.subtract)
                # tree-max margins over rounds -> ACC
                width, src, flip = wb, tt, 0
                while width > 96:
                    h = width // 2
                    dst = (TR1, TR2)[flip % 2]
                    treng.tensor_tensor(out=dst[:, 0:h], in0=src[:, 0:h],
                                        in1=src[:, h:2 * h], op=OP.max)
                    src, width, flip = dst, h, flip + 1
                acc_sl = ACC[:, im * 96:(im + 1) * 96]
                treng.tensor_tensor(out=acc_sl, in0=acc_sl, in1=src[:, 0:96],
                                    op=OP.max)
            # notign = (max margin <= athp) for this image
            nc.vector.tensor_tensor(out=MASK[:, im * 96:(im + 1) * 96],
                                    in0=ACC[:, im * 96:(im + 1) * 96],
                                    in1=ATH[:, im * 96:(im + 1) * 96],
                                    op=OP.is_le)

        # conf softplus on ACT while DVE scans
        conf_view = A(RAW, 384, [[960, P], [480, 2], [1, 96]])
        sp_flat = [[192, P], [96, 2], [1, 96]]
        nc.scalar.activation(out=A(SPA, 0, sp_flat), in_=conf_view, func=AF.Abs)
        nc.scalar.activation(out=SPB[:], in_=SPA[:], func=AF.Exp, scale=-1.0)
        nc.scalar.activation(out=SPA[:], in_=SPB[:], func=AF.Ln, bias=1.0)
        nc.scalar.activation(out=A(SPB, 0, sp_flat), in_=conf_view, func=AF.Relu)

        # ---- sparse gt-cell terms (consumers of the early gather) ----
        nc.scalar.activation(out=SPC[0:NGmax, 4:85], in_=GT85[0:NGmax, 4:85],
                             func=AF.Abs)
        nc.scalar.activation(out=SPD[0:NGmax, 4:85], in_=SPC[0:NGmax, 4:85],
                             func=AF.Exp, scale=-1.0)
        nc.scalar.activation(out=SPC[0:NGmax, 4:85], in_=SPD[0:NGmax, 4:85],
                             func=AF.Ln, bias=1.0)
        nc.scalar.activation(out=SPD[0:NGmax, 4:85], in_=GT85[0:NGmax, 4:85],
                             func=AF.Relu)
        nc.vector.tensor_tensor(out=OC[0:NGmax, 4:85], in0=GT85[0:NGmax, 4:85],
                                in1=A(CF, off_oh + 4, [[CWF, NGmax], [1, 81]]),
                                op=OP.mult)
        nc.vector.tensor_tensor(out=U[0:NGmax, 4:85], in0=SPC[0:NGmax, 4:85],
                                in1=SPD[0:NGmax, 4:85], op=OP.add)
        nc.vector.tensor_tensor(out=U[0:NGmax, 4:85], in0=U[0:NGmax, 4:85],
                                in1=OC[0:NGmax, 4:85], op=OP.subtract)
        nc.vector.tensor_tensor(out=OC[0:NGmax, 0:4], in0=GT85[0:NGmax, 0:4],
                                in1=A(CF, off_tgt, [[CWF, NGmax], [1, 4]]),
                                op=OP.subtract)
        nc.scalar.activation(out=U[0:NGmax, 0:4], in_=OC[0:NGmax, 0:4],
                             func=AF.Square)
        # ---- tail: only ops that depend on the scan or the bbox Square ----
        nc.vector.tensor_tensor(out=U[0:NGmax, :], in0=U[0:NGmax, :],
                                in1=A(CF, off_vn, [[CWF, NGmax], [1, 85]]),
                                op=OP.mult)
        nc.vector.reduce_sum(out=OUTS[0:NGmax, 2:3], in_=U[0:NGmax, :],
                             axis=AX.X)
        nc.vector.tensor_tensor(out=SP[:], in0=SPA[:], in1=SPB[:], op=OP.add)
        nc.vector.tensor_tensor(out=MEXCL[:], in0=SP[:], in1=NGM[:],
                                op=OP.mult)
        for im in (1, 0):
            sl96 = slice(im * 96, im * 96 + 96)
            nc.vector.tensor_tensor(out=SPM[:, sl96], in0=MEXCL[:, sl96],
                                    in1=MASK[:, sl96], op=OP.mult)
            nc.vector.reduce_sum(out=OUTS[:, im:im + 1], in_=SPM[:, sl96],
                                 axis=AX.X)

        nc.sync.dma_start(out=out[:], in_=OUTS[:])

    return nc


_CACHE = {}
TRACE = False
LAST_RESULTS = None


def _split_multiwait(nc):
    """Walrus codegen on this toolchain supports only one sync-wait command
    per instruction; split multi-wait instructions into single-wait NOPs on
    the same engine."""
    import concourse.mybir as mybir

    if getattr(nc, "_fcos_wait_split", False):
        return
    nc._fcos_wait_split = True
    for bb in nc.m.functions[0].blocks:
        insts = bb.instructions
        for ins in list(insts):
            si = ins.sync_info
            if si is not None and len(si.on_wait) > 1:
                waits = list(si.on_wait)
                idx = insts.index(ins)
                nops = []
                for j, w in enumerate(waits[:-1]):
                    nop = mybir.InstNoOp(name=f"{ins.name}-wsplit{j}", ins=[],
                                         outs=[])
                    nop.engine = ins.engine
                    nop.sync_info = mybir.SyncInfo(on_wait=[w], on_update=[])
                    nops.append(nop)
                ins.sync_info = mybir.SyncInfo(on_wait=[waits[-1]],
                                               on_update=list(si.on_update))
                for nop in reversed(nops):
                    insts.insert(idx, nop)


def kernel(raw, labels, anchors_all, img_size):
    from concourse.bass_utils import run_bass_kernel_spmd

    raw = np.asarray(raw, f32)
    labels_np = np.asarray(labels, f32)
    anchors_np = np.asarray(anchors_all, f32)
    isize = int(img_size)

    per_img, A_imgs, B_imgs, GA, GB, NGmax = _plan(labels_np, anchors_np, isize)
    key = (GA, GB, NGmax, DUP, GPSIMD_TREE, anchors_np.tobytes(), isize)
    if key not in _CACHE:
        _CACHE[key] = _build_program(GA, GB, NGmax, anchors_np.tolist(), isize)
    nc = _CACHE[key]
    _split_multiwait(nc)

    in_maps = [
        _pack_core_inputs(c, per_img, A_imgs, B_imgs, raw, isize, GA, GB, NGmax)
        for c in range(N_CORES)
    ]
    global LAST_RESULTS
    res = run_bass_kernel_spmd(nc, in_maps, list(range(N_CORES)), trace=TRACE)
    LAST_RESULTS = res
    total = np.float64(0.0)
    for c in range(N_CORES):
        o = res.results[c]["out"]
        total += np.sum(o[:, 0:3], dtype=np.float64)
    return f32(total)


if __name__ == "__main__":
    import importlib.util

    spec = importlib.util.spec_from_file_location("reference",
                                                  "/root/problem/reference.py")
    ref = importlib.util.module_from_spec(spec)
    spec.loader.exec_module(ref)
    inputs = ref.setup_inputs()
    np_inputs = {k: np.asarray(v) for k, v in inputs.items()}
    got = kernel(**np_inputs)
    print("kernel:", got)
